# revision 8
# baseline (speedup 1.0000x reference)
"""Trainium2 Bass kernel for nn_DenseAttentionMultiHead (dense_transformer).

Reference (B=4, S=2048, H=2048, 16 heads, D=128, no softmax):
    x   = rope(hidden_states)                       # [B, S, H]
    q   = x @ W.T
    out = ((q_h @ k_h^T) @ k_h  per head)           # k == roped x heads

Algebraic rewrite (no softmax): (Q K^T) K == Q (K^T K), so the [S,S]
intermediate collapses to a [128,128] Gram matrix per head.

Sharding (8 cores): core c -> (batch b = c // 2, head-group g = c % 2).
Outputs are disjoint -> no collectives. The host rolls the H axis so each
core's own heads sit in columns 1024..2047 (one SPMD program for both
head-groups; the q-projection contraction is permutation invariant).

v2 layout (vs the h-major baseline): x is loaded in natural [S, H] layout
(s on partitions). Rope pairs (d, d+64) are then FREE-AXIS neighbours, so
rope is pure DVE work with AP tricks (no cross-partition ACT copies), and
the per-head Gram contracts over s = partitions directly from the roped
tile (no DMA transpose for the Gram). One XBAR transpose per s-tile
produces the [H, S] layout the q-projection needs. All inputs host-cast
to bf16 (halves load traffic; no SWDGE casting descriptors).

Schedule: per s-tile: load (SWDGE) -> rope (DVE: 2 half muls + full mul +
add) -> 8 Gram matmuls (two PSUM banks, 4 heads per bank via has_written
semantics) -> XBAR transpose (sync ring only, so qt copies on ACT never
delay transposes). Every 4 tiles, one full-k q^T s-chunk (8 m-tiles x 16
k-tiles of N=512 matmuls) streams on the PE. At the last chunk, per
m-tile: q matmuls, qt copy, Gram->SBUF copy, and the 4 out matmuls + bf16
output DMA, so the tail is ~2us.
"""

from contextlib import ExitStack

import numpy as np

import concourse.bass as bass
import concourse.tile as tile
from concourse import bacc, mybir
from concourse.bass import ts
from concourse.bass_utils import run_bass_kernel_spmd

B, S, H = 4, 2048, 2048
NH, D = 16, 128
HPC = 8  # heads per core
GCOLS = HPC * D  # 1024 q-columns per core
NT = S // 128  # 16 s-tiles
NMT = 8  # q column tiles (== heads per core)
SC = 512
NSC = S // SC  # 4
F32 = mybir.dt.float32
BF16 = mybir.dt.bfloat16


def v3(ap):
    """[128, 2048] -> [128, 16, 128] head-blocked view."""
    return ap.rearrange("p (a b) -> p a b", a=NH)


def build_kernel(ctx: ExitStack, tc: tile.TileContext, xs, wk, ccx, ssx, outT):
    nc = tc.nc

    p_x = ctx.enter_context(tc.tile_pool(name="xpool", bufs=8))
    p_tmp = ctx.enter_context(tc.tile_pool(name="tmp", bufs=1))
    p_xr = ctx.enter_context(tc.tile_pool(name="xr", bufs=6))
    p_xrT = ctx.enter_context(tc.tile_pool(name="xrT", bufs=NSC))
    p_w = ctx.enter_context(tc.tile_pool(name="wpool", bufs=NMT))
    p_cs = ctx.enter_context(tc.tile_pool(name="cs", bufs=2))
    p_qt = ctx.enter_context(tc.tile_pool(name="qt", bufs=NMT))
    p_ms = ctx.enter_context(tc.tile_pool(name="ms", bufs=2))
    p_ot = ctx.enter_context(tc.tile_pool(name="ot", bufs=2))
    ps_g = ctx.enter_context(tc.tile_pool(name="psg", bufs=2, space="PSUM"))
    ps_mm = ctx.enter_context(tc.tile_pool(name="psmm", bufs=6, space="PSUM"))

    # rope coefficient tables, pre-expanded on host: CC = [cos|cos],
    # SS = [-sin|sin], laid out [128, s-tile, 128]
    cc3 = p_cs.tile([128, NT, 128], BF16, tag="cc")
    nc.gpsimd.dma_start(out=cc3[:], in_=ccx[:])
    ss3 = p_cs.tile([128, NT, 128], BF16, tag="ss")
    nc.gpsimd.dma_start(out=ss3[:], in_=ssx[:])

    g0 = ps_g.tile([128, 512], F32, tag="g", name="g0")
    g1 = ps_g.tile([128, 512], F32, tag="g", name="g1")

    wt = [p_w.tile([128, NT, 128], BF16, tag="wb", name=f"wb{m}") for m in range(NMT)]
    xrT = []
    qt = [p_qt.tile([128, S], BF16, tag="qt", name=f"qt{m}") for m in range(NMT)]

    # ---- load plan: sync ring carries ONLY x0/x1 then the 16 transposes
    # (transposes serialize against in-flight DMAs, so they get their own
    # ring and early slots). scalar: early x + W strips 0-3. gpsimd: coeffs,
    # W strips 4-7, bulk x. Later x tiles are emitted in-loop so the 9-buf
    # x pool recycles.
    xtiles = [p_x.tile([128, S], BF16, tag="xb", name=f"xb{t}") for t in range(NT)]

    def load_x(t, eng):
        eng.dma_start(out=xtiles[t][:], in_=xs[ts(t, 128), :])

    load_x(0, nc.sync)
    load_x(1, nc.sync)
    for t in (2, 3, 4, 5):
        load_x(t, nc.scalar)
    for m in range(4, NMT):
        nc.gpsimd.dma_start(out=wt[m][:], in_=wk[m])
    for m in (0, 1):
        nc.scalar.dma_start(out=wt[m][:], in_=wk[m])
    for t in (6, 7):
        load_x(t, nc.scalar)
    for m in (2, 3):
        nc.scalar.dma_start(out=wt[m][:], in_=wk[m])
    load_x(8, nc.gpsimd)

    for t in range(NT):
        if 5 <= t <= 11:  # x9..x15 stream behind freed buffers
            load_x(t + 4, nc.gpsimd)
        xb = xtiles[t]

        # ---- rope (all DVE): xr = x*CC + shift64(x)*SS
        xv = v3(xb[:])
        ccB = cc3[:, t : t + 1, :].broadcast_to([128, NH, 128])
        ssL = ss3[:, t : t + 1, 0:64].broadcast_to([128, NH, 64])
        ssR = ss3[:, t : t + 1, 64:128].broadcast_to([128, NH, 64])
        tmp = p_tmp.tile([128, S], BF16, tag="tmp", name=f"tmp{t}")
        tv = v3(tmp[:])
        nc.vector.tensor_mul(tv[:, :, 0:64], xv[:, :, 64:128], ssL)
        nc.vector.tensor_mul(tv[:, :, 64:128], xv[:, :, 0:64], ssR)
        xr = p_xr.tile([128, S], BF16, tag="xr", name=f"xr{t}")
        xrv = v3(xr[:])
        nc.vector.tensor_mul(xrv, xv, ccB)
        nc.vector.tensor_add(xr[:], xr[:], tmp[:])

        # ---- Gram accumulation: own heads live in columns 1024+h*128.
        # 4 heads share a PSUM bank; the single start=True clears the whole
        # bank's has_written bits, later quarters overwrite-where-unset.
        for h in range(HPC):
            sl = xrv[:, HPC + h, :]
            bank = g0 if h < 4 else g1
            gq = bank[:, ts(h % 4, 128)]
            nc.tensor.matmul(
                gq,
                sl,
                sl,
                start=(t == 0 and h % 4 == 0),
                stop=(t == NT - 1),
                skip_group_check=True,
            )

        # ---- XBAR transpose into the [H, S] operand for the q-projection
        if t % 4 == 0:
            xrT.append(
                p_xrT.tile([128, NT, SC], BF16, tag="xrT", name=f"xrT{t // 4}")
            )
        nc.sync.dma_start_transpose(
            xrT[t // 4][:, :, ts(t % 4, 128)], xr[:]
        )

        # ---- q-projection for the completed s-chunk
        if t % 4 == 3:
            sc = t // 4
            last = sc == NSC - 1
            for m in range(NMT):
                ps = ps_mm.tile([128, SC], F32, tag="mm", name=f"q{sc}_{m}")
                wv = wt[m][:]
                for kt in range(NT):
                    nc.tensor.matmul(
                        ps[:],
                        wv[:, kt, :],
                        xrT[sc][:, kt, :],
                        start=(kt == 0),
                        stop=(kt == NT - 1),
                    )
                nc.scalar.copy(qt[m][:, ts(sc, SC)], ps[:])
                if last:
                    # Gram -> SBUF, then this head's 4 out-matmuls ride
                    # right behind its final q-chunk (short tail).
                    msb = p_ms.tile([128, 128], BF16, tag="ms", name=f"ms{m}")
                    bank = g0 if m < 4 else g1
                    nc.vector.tensor_copy(msb[:], bank[:, ts(m % 4, 128)])
                    for oc in range(NSC):
                        ops = ps_mm.tile(
                            [128, SC], F32, tag="mm", name=f"o{m}_{oc}"
                        )
                        nc.tensor.matmul(
                            ops[:],
                            msb[:],
                            qt[m][:, ts(oc, SC)],
                            start=True,
                            stop=True,
                        )
                        ot = p_ot.tile([128, SC], BF16, tag="ot", name=f"ot{m}_{oc}")
                        if oc % 2 == 0:
                            nc.scalar.copy(ot[:], ops[:])
                        else:
                            nc.vector.tensor_copy(ot[:], ops[:])
                        nc.gpsimd.dma_start(
                            out=outT[ts(m, 128), ts(oc, SC)], in_=ot[:]
                        )


_NC_CACHE = {}


def build_nc():
    if "nc" in _NC_CACHE:
        return _NC_CACHE["nc"]
    nc = bacc.Bacc("TRN2", target_bir_lowering=False, debug=False)
    xs = nc.dram_tensor("xs", [S, H], BF16, kind="ExternalInput").ap()
    wk = nc.dram_tensor("wk", [NMT, 128, NT, 128], BF16, kind="ExternalInput").ap()
    ccx = nc.dram_tensor("ccx", [128, NT, 128], BF16, kind="ExternalInput").ap()
    ssx = nc.dram_tensor("ssx", [128, NT, 128], BF16, kind="ExternalInput").ap()
    outT = nc.dram_tensor("outT", [GCOLS, S], BF16, kind="ExternalOutput").ap()
    with tile.TileContext(nc) as tc:
        with ExitStack() as ctx:
            build_kernel(ctx, tc, xs, wk, ccx, ssx, outT)
    nc.compile()
    _NC_CACHE["nc"] = nc
    return nc


def make_in_maps(hidden_states, W, cos, sin):
    """Host prep: bf16 casts, head-roll so own heads sit at cols 1024+,
    strip-major W layout, pre-expanded rope tables."""
    bf16 = mybir.dt.np(BF16)
    x = np.asarray(hidden_states, dtype=np.float32)
    W_ = np.asarray(W, dtype=np.float32)
    cos = np.asarray(cos, dtype=np.float32)
    sin = np.asarray(sin, dtype=np.float32)

    CC = np.concatenate([cos, cos], axis=1)  # [S, 128]
    SS = np.concatenate([-sin, sin], axis=1)
    cc3 = np.ascontiguousarray(
        CC.reshape(NT, 128, 128).transpose(1, 0, 2)
    ).astype(bf16)
    ss3 = np.ascontiguousarray(
        SS.reshape(NT, 128, 128).transpose(1, 0, 2)
    ).astype(bf16)

    in_maps = []
    for c in range(8):
        b, g = c // 2, c % 2
        xb = x[b]  # [S, H]
        Wgt = W_[GCOLS * g : GCOLS * (g + 1), :].T  # [H, 1024]
        if g == 0:  # own heads (cols 0:1024) -> cols 1024:2048
            xb = np.roll(xb, GCOLS, axis=1)
            Wgt = np.roll(Wgt, GCOLS, axis=0)
        # strips: wk[m][p, kt, j] = Wgt[kt*128+p, m*128+j]
        wkm = np.ascontiguousarray(
            Wgt.reshape(NT, 128, NMT, 128).transpose(2, 1, 0, 3)
        ).astype(bf16)
        in_maps.append(
            {
                "xs": np.ascontiguousarray(xb).astype(bf16),
                "wk": wkm,
                "ccx": cc3,
                "ssx": ss3,
            }
        )
    return in_maps


def run(hidden_states, W, cos, sin, trace=False):
    nc = build_nc()
    in_maps = make_in_maps(hidden_states, W, cos, sin)
    res = run_bass_kernel_spmd(nc, in_maps, list(range(8)), trace=trace)
    out = np.empty((B, S, H), np.float32)
    for c in range(8):
        b, g = c // 2, c % 2
        out[b][:, GCOLS * g : GCOLS * (g + 1)] = (
            res.results[c]["outT"].astype(np.float32).T
        )
    return out, res


def kernel(hidden_states, W, cos, sin):
    out, _ = run(hidden_states, W, cos, sin, trace=False)
    return out
